# revision 10
# baseline (speedup 1.0000x reference)
"""Multi-head causal attention (B=4, T=2048, D=1024, H=16, Dh=64) on 8 trn2 cores.

Sharding: 4-way DP over batch x 2-way TP over heads.
Core c handles batch c//2 and heads (c%2)*8 .. (c%2)*8+7.
Each core computes a partial output [T, D] (its heads' contribution through
w_out rows); host sums the two partials per batch.

Per-core device kernel (bf16 matmul operands, fp32 PSUM accumulation):
  v[t, f]   = sum_d xT[d, t] * w_v[d, f]      (v in [tok, feat] layout,
                                               + fused ones column per head)
  qkT[f, t] = sum_d w_qk[d, f] * xT[d, t]     (q/k in [feat, tok] layout)
  attention, q-block j OUTER, head-pair hp inner, 2 k-tiles per period:
      S^T[k, q] = sum_d kT[d, k] * qT[d, q]   (row-split PE mode: the two
                                               heads of a pair use disjoint
                                               64-row PE groups, concurrent)
      P^T = exp(S^T / 8)                      (ACT; no max-subtraction)
      causal mask on diagonal k-tiles via gpsimd affine_select
      o^T[m, q] = sum_k v_aug[k, m] * P^T[k, q]   (m: 64 v-feats + ones row
                                                   -> row 64 = denominator)
      attn^T = o^T[0:64] * recip(o^T[64]) broadcast via gpsimd
               partition_broadcast (PE rank-1 matmul for the final pair,
               where the PE is idle and latency is king)
  y[t, n] = sum_f attn^T[f, t] * w_o[f, n]

Scheduling (v3): periods batch TWO k-tiles of S^T (split-mode span) before
switching the PE back to normal mode for filler/PV work -- each
split<->normal transition costs ~100ns of PE drain. Projection groups pop
from a deadline-ordered queue (2 periods before consumption);
out-projection groups are slack-scheduled into the late blocks where the
ACT exp throughput (~1ns/col) would otherwise outpace the lean S^T+PV
stream, idle the PE, and make HAM throttle the clock. Input DMAs are many
fine-grained contiguous blocks (host-side repack) so the 16 DMA engines
run in parallel; late, non-critical DMAs are issued from inside the period
loop to keep the issuing queues clear. PE warm-up matmuls bridge the ~8us
engine bootstrap to the first data-dependent work for the HAM ramp.
"""

import numpy as np
import ml_dtypes

import concourse.mybir as mybir
import concourse.tile as tile
from concourse import bacc, bass_utils

F32 = mybir.dt.float32
BF16 = mybir.dt.bfloat16

D = 1024          # model dim
T = 2048          # tokens per batch
DH = 64           # head dim
NH_LOC = 8        # heads per core
DT = D // 128     # D tiles (contraction)
TT = T // 128     # token tiles
QB = T // 512     # q blocks of 512
VW = DH + 1       # v width incl ones column
NWARM = 10        # HAM warm-up matmuls

# period index bookkeeping: block j has 4 hps x (j+1) two-kt periods
STARTS = [0, 8, 24, 48]


def hp_start(j, hp):
    return STARTS[j] + hp * 2 * (j + 1)


def build_kernel():
    nc = bacc.Bacc()
    # DRAM layouts are host-side block-packed so every DMA below is one
    # fully contiguous DRAM read:
    #  xT: (tb, dt, p, c) -> per-(tb,dt) [128,512] 128KB blocks
    #  w_qk: (f, dt, p, c) -> per-(f, dt-half) [512,128] 128KB blocks
    #  w_v: natural (dt*128+p, c) -> per-dt 128KB blocks
    #  w_o: natural (hp4*128+p, c) -> per-hp4 256KB blocks
    xT_d = nc.dram_tensor("xT", [4096, 512], BF16, kind="ExternalInput")
    wqk_d = nc.dram_tensor("w_qk", [8192, 128], BF16, kind="ExternalInput")
    wv_d = nc.dram_tensor("w_v", [1024, 512], BF16, kind="ExternalInput")
    wo_d = nc.dram_tensor("w_o", [512, 1024], BF16, kind="ExternalInput")
    y_d = nc.dram_tensor("y", [T, D], F32, kind="ExternalOutput")

    with tile.TileContext(nc) as tc:
        with (
            tc.tile_pool(name="big", bufs=1) as big,
            tc.tile_pool(name="ptp", bufs=6) as ptp,
            tc.tile_pool(name="ovp", bufs=8) as ovp,
            tc.tile_pool(name="stg", bufs=2) as stg,
            tc.tile_pool(name="ps_st", bufs=2, space="PSUM") as ps_st,
            tc.tile_pool(name="ps_pv", bufs=2, space="PSUM") as ps_pv,
            tc.tile_pool(name="ps_mm", bufs=2, space="PSUM") as ps_mm,
        ):
            xt_all = big.tile([128, DT, 2048], BF16, tag="xt")
            wqk_all = big.tile([128, DT, 1024], BF16, tag="wqk")
            wv_all = big.tile([128, DT, 512], BF16, tag="wv")
            wo_all = big.tile([128, 4, 1024], BF16, tag="wo")
            qk = [big.tile([128, T], BF16, tag=f"qk{i}", name=f"qk{i}") for i in range(8)]
            attn_t = [big.tile([128, T], BF16, tag=f"attn{i}", name=f"attn{i}") for i in range(4)]
            vsb_t = [big.tile([128, 2, NH_LOC * VW], BF16, tag=f"vsb{i}", name=f"vsb{i}") for i in range(8)]
            warm = big.tile([1, 512], BF16, tag="warm")
            ones = big.tile([1, DH], BF16, tag="ones")
            vsb_r = [t.rearrange("p t (h c) -> p t h c", c=VW) for t in vsb_t]

            # ---- HAM warm-up: PE activity from the end of engine bootstrap
            # (~8us) until the first DMA-fed groups are ready ----
            nc.vector.memset(warm, 1.0)
            nc.vector.memset(ones, 1.0)
            ps_w = ps_mm.tile([128, 512], F32, tag="mm")
            for _ in range(NWARM):
                nc.tensor.matmul(ps_w[0:1, 0:256], lhsT=warm[0:1, 0:1],
                                 rhs=warm[0:1, 0:256], start=True, stop=True)

            # ---- DMA emitters ----
            def dma_xt(tb, dt, eng):
                r0 = (tb * 8 + dt) * 128
                eng.dma_start(
                    xt_all[:, dt, tb * 512:(tb + 1) * 512],
                    xT_d[r0:r0 + 128, :],
                )

            def dma_wqk(f, half, eng):
                r0 = f * 1024 + half * 512
                eng.dma_start(
                    wqk_all[:, half * 4:(half + 1) * 4, f * 128:(f + 1) * 128],
                    wqk_d[r0:r0 + 512, :].rearrange("(d p) c -> p d c", d=4),
                )

            def dma_wv(dt, eng):
                eng.dma_start(
                    wv_all[:, dt, :],
                    wv_d[dt * 128:(dt + 1) * 128, :],
                )

            def dma_wo(hp4, eng):
                eng.dma_start(
                    wo_all[:, hp4, :],
                    wo_d[hp4 * 128:(hp4 + 1) * 128, :],
                )

            # critical first wave: everything block j0 touches, spread over
            # the three DMA-issuing queues
            for dt in range(DT):
                dma_xt(0, dt, nc.sync)
            for f in (0, 4, 1, 5, 2, 6):
                for half in range(2):
                    dma_wqk(f, half, nc.scalar)
            for dt in range(DT):
                dma_wv(dt, nc.gpsimd)
            for f in (3, 7):
                for half in range(2):
                    dma_wqk(f, half, nc.gpsimd)
            for dt in range(DT):
                dma_xt(1, dt, nc.sync)

            # late DMAs issue from inside the period loop (keeps the issuing
            # queues clear for affine_select / exp / y writes early on)
            dma_q = []
            for dt in range(DT):
                dma_q.append((5, lambda dt=dt: dma_xt(2, dt, nc.sync)))
            for hp4 in range(4):
                dma_q.append((6, lambda hp4=hp4: dma_wo(hp4, nc.gpsimd)))
            for dt in range(DT):
                dma_q.append((16, lambda dt=dt: dma_xt(3, dt, nc.sync)))

            # ---- projection group emitters ----
            def v_group(tt):
                def go():
                    ps = ps_mm.tile([128, 512], F32, tag="mm")
                    for dt in range(DT):
                        nc.tensor.matmul(
                            ps,
                            lhsT=xt_all[:, dt, tt * 128:(tt + 1) * 128],
                            rhs=wv_all[:, dt, :],
                            start=(dt == 0),
                            stop=(dt == DT - 1),
                        )
                    nc.vector.tensor_copy(
                        vsb_r[tt // 2][:, tt % 2, :, 0:DH],
                        ps.rearrange("p (h c) -> p h c", c=DH),
                    )
                    nc.vector.memset(vsb_r[tt // 2][:, tt % 2, :, DH], 1.0)
                return go

            def qk_group(f, tb):
                def go():
                    ps = ps_mm.tile([128, 512], F32, tag="mm")
                    for dt in range(DT):
                        nc.tensor.matmul(
                            ps,
                            lhsT=wqk_all[:, dt, f * 128:(f + 1) * 128],
                            rhs=xt_all[:, dt, tb * 512:(tb + 1) * 512],
                            start=(dt == 0),
                            stop=(dt == DT - 1),
                        )
                    nc.vector.tensor_copy(qk[f][:, tb * 512:(tb + 1) * 512], ps)
                return go

            def out_group(tt, nb):
                def go():
                    ps = ps_mm.tile([128, 512], F32, tag="mm")
                    for hp4 in range(4):
                        nc.tensor.matmul(
                            ps,
                            lhsT=attn_t[hp4][:, tt * 128:(tt + 1) * 128],
                            rhs=wo_all[:, hp4, nb * 512:(nb + 1) * 512],
                            start=(hp4 == 0),
                            stop=(hp4 == 3),
                        )
                    ysb = stg.tile([128, 512], F32, tag="y", bufs=4,
                                   name=f"ysb{tt}_{nb}")
                    nc.vector.tensor_copy(ysb, ps)
                    nc.sync.dma_start(
                        y_d[tt * 128:(tt + 1) * 128, nb * 512:(nb + 1) * 512],
                        ysb,
                    )
                return go

            # ---- deadline filler queue (projection groups) ----
            deadline_q = []
            for tt in (0, 1):
                deadline_q.append((0, v_group(tt)))
            deadline_q.append((0, qk_group(1, 0)))
            deadline_q.append((0, qk_group(5, 0)))
            for tt in (2, 3):
                deadline_q.append((1, v_group(tt)))
            for j in range(QB):
                for hp in range(4):
                    if j == 0 and hp in (0, 1):
                        continue  # upfront / added above
                    dl = hp_start(j, hp) - 2
                    deadline_q.append((dl, qk_group(hp, j)))
                    deadline_q.append((dl, qk_group(4 + hp, j)))
            for tt in range(4, TT):
                jb = tt // 4
                deadline_q.append((STARTS[jb] + tt // 2 - 1, v_group(tt)))
            deadline_q.sort(key=lambda e: e[0])

            slack_q = []          # (earliest_period, fn, block_j)
            out_ready = [False] * QB
            stages = []           # deferred epilogue stages (None = spacer)

            def period_extras(P):
                while dma_q and dma_q[0][0] <= P:
                    dma_q.pop(0)[1]()
                while deadline_q and deadline_q[0][0] <= P:
                    deadline_q.pop(0)[1]()
                npop = 2 if (len(stages) > 5 or P >= 56) else 1
                for _ in range(npop):
                    if stages:
                        s = stages.pop(0)
                        if s is not None:
                            s()
                if slack_q and slack_q[0][0] <= P and out_ready[slack_q[0][2]]:
                    slack_q.pop(0)[1]()

            def push_epilogue(hp, j, pvA, pvB):
                # free the PSUM accumulators right away (bf16 SBUF copies);
                # denominators (row 64) concatenated into one [1,1024] f32
                ova = ovp.tile([64, 512], BF16, tag="ov", name=f"ova{hp}_{j}")
                ovb = ovp.tile([64, 512], BF16, tag="ov", name=f"ovb{hp}_{j}")
                dn = stg.tile([1, 1024], F32, tag="dn", name=f"dn{hp}_{j}")
                nc.vector.tensor_copy(ova, pvA[0:DH, :])
                nc.vector.tensor_copy(ovb, pvB[0:DH, :])
                nc.vector.tensor_copy(dn[:, 0:512], pvA[DH:DH + 1, :])
                nc.vector.tensor_copy(dn[:, 512:1024], pvB[DH:DH + 1, :])

                def stage1():
                    rec = stg.tile([1, 1024], F32, tag="rec", name=f"rec{hp}_{j}")
                    rb = stg.tile([1, 1024], BF16, tag="rb", name=f"rb{hp}_{j}")
                    nc.vector.reciprocal_approx_fast(out=rec, in_=dn)
                    nc.vector.tensor_copy(rb, rec)
                    stage1.rb = rb

                jc = slice(j * 512, (j + 1) * 512)

                def stage2():
                    # rank-1 PE broadcast of the reciprocals (the gpsimd
                    # partition_broadcast shares a FIFO with the critical
                    # affine_select masks and forces a library reload)
                    bc = ps_mm.tile([128, 512], F32, tag="mm")
                    nc.tensor.matmul(bc[0:DH, :], lhsT=ones,
                                     rhs=stage1.rb[:, 0:512],
                                     start=True, stop=True)
                    nc.tensor.matmul(bc[64:64 + DH, :], lhsT=ones,
                                     rhs=stage1.rb[:, 512:1024],
                                     start=True, stop=True)
                    nc.vector.tensor_mul(attn_t[hp][0:64, jc], ova,
                                         bc[0:DH, :])
                    nc.vector.tensor_mul(attn_t[hp][64:128, jc], ovb,
                                         bc[64:64 + DH, :])
                    if hp == 3:
                        out_ready[j] = True

                stages.extend([stage1, None, stage2])

            # up-front: only what attention period 0 needs
            qk_group(0, 0)()
            qk_group(4, 0)()

            # ---- attention: q-block j OUTER, head-pair inner, 2 k-tiles
            # per period. The two heads of a pair sit on partitions 0-63 /
            # 64-127 of the same qk tiles, so their K=64 S^T matmuls go to
            # disjoint PE row groups and run concurrently. ----
            P = 0
            for j in range(QB):
                for hp in range(4):
                    qTf = qk[hp]
                    kTf = qk[4 + hp]
                    nkt = 4 * (j + 1)
                    pvA = ps_pv.tile([128, 512], F32, tag="pv")
                    pvB = ps_pv.tile([128, 512], F32, tag="pv")
                    pv_queue = []

                    def pv_mms(kt, pt, q0, pvA=pvA, pvB=pvB, hp=hp, nkt=nkt):
                        def go():
                            nc.tensor.matmul(
                                pvA[0:VW, q0:512],
                                lhsT=vsb_r[kt // 2][:, kt % 2, 2 * hp, :],
                                rhs=pt[:, q0:512],
                                start=(kt == 0), stop=(kt == nkt - 1),
                            )
                            nc.tensor.matmul(
                                pvB[0:VW, q0:512],
                                lhsT=vsb_r[kt // 2][:, kt % 2, 2 * hp + 1, :],
                                rhs=pt[:, 512 + q0:1024],
                                start=(kt == 0), stop=(kt == nkt - 1),
                            )
                        return go

                    def emit_st(kt, j=j, hp=hp, qTf=qTf, kTf=kTf):
                        # diagonal k-tiles: q < 128*(kt-4j) is fully masked --
                        # narrow S^T/exp/mask/PV to the live columns
                        q0 = 128 * (kt - 4 * j) if kt >= 4 * j else 0
                        nq = 512 - q0
                        st = ps_st.tile([128, 1024], F32, tag="st")
                        nc.tensor.matmul(
                            st[:, q0:512],
                            lhsT=kTf[0:64, kt * 128:(kt + 1) * 128],
                            rhs=qTf[0:64, j * 512 + q0:(j + 1) * 512],
                            start=True, stop=True,
                        )
                        nc.tensor.matmul(
                            st[:, 512 + q0:1024],
                            lhsT=kTf[64:128, kt * 128:(kt + 1) * 128],
                            rhs=qTf[64:128, j * 512 + q0:(j + 1) * 512],
                            start=True, stop=True,
                        )
                        pt = ptp.tile([128, 1024], BF16, tag="pt",
                                      name=f"pt{hp}_{j}_{kt}")
                        st_r = st.rearrange("p (h q) -> p h q", h=2)
                        pt_r = pt.rearrange("p (h q) -> p h q", h=2)
                        nc.scalar.activation(
                            pt_r[:, :, q0:512], st_r[:, :, q0:512],
                            mybir.ActivationFunctionType.Exp, scale=0.125
                        )
                        if kt >= 4 * j:  # diagonal k-tile: zero where k > q
                            for half in range(2):
                                nc.gpsimd.affine_select(
                                    out=pt[:, half * 512 + q0:(half + 1) * 512],
                                    in_=pt[:, half * 512 + q0:(half + 1) * 512],
                                    compare_op=mybir.AluOpType.is_ge,
                                    fill=0.0,
                                    base=0,
                                    pattern=[[1, nq]],
                                    channel_multiplier=-1,
                                )
                        return pt, q0

                    for kp in range(nkt // 2):
                        last = kp == nkt // 2 - 1
                        if not last:
                            period_extras(P)
                        for kt in (2 * kp, 2 * kp + 1):
                            pt, q0 = emit_st(kt)
                            pv_queue.append(pv_mms(kt, pt, q0))
                        if last:
                            # flush period: filler between S^T and the full
                            # PV drain gives the last exps time to land
                            period_extras(P)
                            for f_ in pv_queue:
                                f_()
                            pv_queue.clear()
                        else:
                            while len(pv_queue) > 2:
                                pv_queue.pop(0)()
                        P += 1

                    if j == 3 and hp == 3:
                        # drain pending epilogues; start the final pair's
                        # copies+reciprocal immediately; fill its latency
                        # window with held-back out groups (keeps the PE busy
                        # so HAM doesn't throttle); then broadcast+multiply
                        # and the last out-projections
                        while stages:
                            s = stages.pop(0)
                            if s is not None:
                                s()
                        push_epilogue(hp, j, pvA, pvB)
                        stages.pop(0)()      # stage1: recip chain
                        stages.pop(0)        # spacer
                        for _, fn, _ in slack_q:
                            fn()
                        slack_q.clear()
                        stages.pop(0)()      # stage2: broadcast + multiply
                    else:
                        push_epilogue(hp, j, pvA, pvB)
                    if hp == 3:
                        if j < 3:
                            # out(2)'s last 4 groups are held for the final
                            # chain window (earliest=999 keeps them queued)
                            earliest = {0: 12, 1: 44, 2: 60}[j]
                            for i, (tt, nb) in enumerate(
                                    (tt, nb)
                                    for tt in range(4 * j, 4 * j + 4)
                                    for nb in range(2)):
                                e = earliest + 2 * i
                                if j == 2 and i >= 4:
                                    e = 999
                                slack_q.append((e, out_group(tt, nb), j))
                        else:
                            for tt in range(12, 16):
                                for nb in range(2):
                                    out_group(tt, nb)()

            while stages:
                s = stages.pop(0)
                if s is not None:
                    s()
            while slack_q:
                slack_q.pop(0)[1]()

    nc.compile()
    return nc


def _shard_inputs(x, w_qkv, w_out):
    """Build the 8 per-core input maps (bf16, DMA-block-packed layouts)."""
    bf16 = ml_dtypes.bfloat16
    in_maps = []
    for c in range(8):
        b = c // 2
        hg = c % 2
        q_cols = slice(hg * 512, hg * 512 + 512)
        k_cols = slice(1024 + hg * 512, 1024 + hg * 512 + 512)
        v_cols = slice(2048 + hg * 512, 2048 + hg * 512 + 512)

        xT = np.ascontiguousarray(x[b].T)                    # [1024, 2048]
        # (tb, dt, p, c) blocks
        x_pack = (xT.reshape(8, 128, 4, 512)                 # (dt,p,tb,c)
                  .transpose(2, 0, 1, 3).reshape(4096, 512))

        w_qk = np.concatenate([w_qkv[:, q_cols], w_qkv[:, k_cols]], axis=1)
        # (f, dt, p, c) blocks
        wqk_pack = (w_qk.reshape(8, 128, 8, 128)             # (dt,p,f,c)
                    .transpose(2, 0, 1, 3).reshape(8192, 128))

        in_maps.append({
            "xT": np.ascontiguousarray(x_pack).astype(bf16),
            "w_qk": np.ascontiguousarray(wqk_pack).astype(bf16),
            "w_v": np.ascontiguousarray(w_qkv[:, v_cols]).astype(bf16),
            "w_o": np.ascontiguousarray(
                w_out[hg * 512:hg * 512 + 512, :]).astype(bf16),
        })
    return in_maps


def _run(inputs, trace=False):
    x = np.asarray(inputs["x"], dtype=np.float32)
    w_qkv = np.asarray(inputs["w_qkv"], dtype=np.float32)
    w_out = np.asarray(inputs["w_out"], dtype=np.float32)
    nc = build_kernel()
    in_maps = _shard_inputs(x, w_qkv, w_out)
    res = None
    for attempt in range(3):
        try:
            res = bass_utils.run_bass_kernel_spmd(
                nc, in_maps, core_ids=list(range(8)), trace=trace
            )
            break
        except Exception:
            if attempt == 2:
                raise
    assert res is not None
    out = np.empty((4, T, D), dtype=np.float32)
    for b in range(4):
        out[b] = res.results[2 * b]["y"] + res.results[2 * b + 1]["y"]
    return out, res


def kernel(**inputs):
    out, _ = _run(inputs, trace=False)
    return out


# revision 15
# speedup vs baseline: 1.1034x; 1.1034x over previous
"""Multi-head causal attention (B=4, T=2048, D=1024, H=16, Dh=64) on 8 trn2 cores.

Sharding: 4-way DP over batch x 2-way TP over heads.
Core c handles batch c//2 and heads (c%2)*8 .. (c%2)*8+7.
Each core computes a partial output [T, D] (its heads' contribution through
w_out rows); host sums the two partials per batch.

Per-core device kernel (bf16 matmul operands, fp32 PSUM accumulation):
  v[t, f]   = sum_d xT[d, t] * w_v[d, f]      (v in [tok, feat] layout,
                                               + fused ones column per head)
  qkT[f, t] = sum_d w_qk[d, f] * xT[d, t]     (q/k in [feat, tok] layout)
  attention, q-block j OUTER, head-pair hp inner, 2 k-tiles per period:
      S^T[k, q] = sum_d kT[d, k] * qT[d, q]   (row-split PE mode: the two
                                               heads of a pair use disjoint
                                               64-row PE groups, concurrent)
      P^T = exp(S^T / 8)                      (ACT; no max-subtraction)
      causal mask on diagonal k-tiles via gpsimd affine_select
      o^T[m, q] = sum_k v_aug[k, m] * P^T[k, q]   (m: 64 v-feats + ones row
                                                   -> row 64 = denominator)
      attn^T = o^T[0:64] * recip(o^T[64]) broadcast via gpsimd
               partition_broadcast (PE rank-1 matmul for the final pair,
               where the PE is idle and latency is king)
  y[t, n] = sum_f attn^T[f, t] * w_o[f, n]

Scheduling (v3): periods batch TWO k-tiles of S^T (split-mode span) before
switching the PE back to normal mode for filler/PV work -- each
split<->normal transition costs ~100ns of PE drain. Projection groups pop
from a deadline-ordered queue (2 periods before consumption);
out-projection groups are slack-scheduled into the late blocks where the
ACT exp throughput (~1ns/col) would otherwise outpace the lean S^T+PV
stream, idle the PE, and make HAM throttle the clock. Input DMAs are many
fine-grained contiguous blocks (host-side repack) so the 16 DMA engines
run in parallel; late, non-critical DMAs are issued from inside the period
loop to keep the issuing queues clear. PE warm-up matmuls bridge the ~8us
engine bootstrap to the first data-dependent work for the HAM ramp.
"""

import numpy as np
import ml_dtypes

import concourse.mybir as mybir
import concourse.tile as tile
from concourse import bacc, bass_utils

F32 = mybir.dt.float32
BF16 = mybir.dt.bfloat16

D = 1024          # model dim
T = 2048          # tokens per batch
DH = 64           # head dim
NH_LOC = 8        # heads per core
DT = D // 128     # D tiles (contraction)
TT = T // 128     # token tiles
QB = T // 512     # q blocks of 512
VW = DH + 1       # v width incl ones column
NWARM = 10        # HAM warm-up matmuls

# period index bookkeeping: block j has 4 hps x (j+1) two-kt periods
STARTS = [0, 8, 24, 48]


def hp_start(j, hp):
    return STARTS[j] + hp * 2 * (j + 1)


def build_kernel():
    nc = bacc.Bacc()
    # DRAM layouts are host-side block-packed so every DMA below is one
    # fully contiguous DRAM read:
    #  xT: (tb, dt, p, c) -> per-(tb,dt) [128,512] 128KB blocks
    #  w_qk: (f, dt, p, c) -> per-(f, dt-half) [512,128] 128KB blocks
    #  w_v: natural (dt*128+p, c) -> per-dt 128KB blocks
    #  w_o: natural (hp4*128+p, c) -> per-hp4 256KB blocks
    xT_d = nc.dram_tensor("xT", [4096, 512], BF16, kind="ExternalInput")
    wqk_d = nc.dram_tensor("w_qk", [8192, 128], BF16, kind="ExternalInput")
    wv_d = nc.dram_tensor("w_v", [1024, 512], BF16, kind="ExternalInput")
    wo_d = nc.dram_tensor("w_o", [512, 1024], BF16, kind="ExternalInput")
    y_d = nc.dram_tensor("y", [T, D], F32, kind="ExternalOutput")

    with tile.TileContext(nc) as tc:
        with (
            tc.tile_pool(name="big", bufs=1) as big,
            tc.tile_pool(name="ptp", bufs=6) as ptp,
            tc.tile_pool(name="ovp", bufs=8) as ovp,
            tc.tile_pool(name="stg", bufs=2) as stg,
            tc.tile_pool(name="ps_st", bufs=2, space="PSUM") as ps_st,
            tc.tile_pool(name="ps_pv", bufs=2, space="PSUM") as ps_pv,
            tc.tile_pool(name="ps_mm", bufs=2, space="PSUM") as ps_mm,
        ):
            xt_all = big.tile([128, DT, 2048], BF16, tag="xt")
            wqk_all = big.tile([128, DT, 1024], BF16, tag="wqk")
            wv_all = big.tile([128, DT, 512], BF16, tag="wv")
            wo_all = big.tile([128, 4, 1024], BF16, tag="wo")
            qk = [big.tile([128, T], BF16, tag=f"qk{i}", name=f"qk{i}") for i in range(8)]
            attn_t = [big.tile([128, T], BF16, tag=f"attn{i}", name=f"attn{i}") for i in range(4)]
            # v operand per head: 64 ONES columns (0:64) then 64 v-feature
            # columns (64:128). PV matmuls are N-bound, so the extra M rows
            # are free -- they deliver the softmax denominator pre-broadcast
            # across 64 PSUM partitions.
            vsb_t = [big.tile([128, 2, NH_LOC * 128], BF16, tag=f"vsb{i}", name=f"vsb{i}") for i in range(8)]
            warm = big.tile([1, 512], BF16, tag="warm")
            vsb_r = [t.rearrange("p t (h c) -> p t h c", c=128) for t in vsb_t]

            # ---- HAM warm-up: PE activity from the end of engine bootstrap
            # (~8us) until the first DMA-fed groups are ready ----
            nc.vector.memset(warm, 1.0)
            for i in range(8):
                nc.vector.memset(vsb_r[i][:, :, :, 0:DH], 1.0)
            ps_w = ps_mm.tile([128, 512], F32, tag="mm")
            for _ in range(NWARM):
                nc.tensor.matmul(ps_w[0:1, 0:256], lhsT=warm[0:1, 0:1],
                                 rhs=warm[0:1, 0:256], start=True, stop=True)

            # ---- DMA emitters ----
            def dma_xt(tb, dt, eng):
                r0 = (tb * 8 + dt) * 128
                eng.dma_start(
                    xt_all[:, dt, tb * 512:(tb + 1) * 512],
                    xT_d[r0:r0 + 128, :],
                )

            def dma_wqk(f, half, eng):
                r0 = f * 1024 + half * 512
                eng.dma_start(
                    wqk_all[:, half * 4:(half + 1) * 4, f * 128:(f + 1) * 128],
                    wqk_d[r0:r0 + 512, :].rearrange("(d p) c -> p d c", d=4),
                )

            def dma_wv(dt, eng):
                eng.dma_start(
                    wv_all[:, dt, :],
                    wv_d[dt * 128:(dt + 1) * 128, :],
                )

            def dma_wo(hp4, eng):
                eng.dma_start(
                    wo_all[:, hp4, :],
                    wo_d[hp4 * 128:(hp4 + 1) * 128, :],
                )

            # critical first wave: everything block j0 touches, spread over
            # the three DMA-issuing queues
            for dt in range(DT):
                dma_xt(0, dt, nc.sync)
            for f in (0, 4, 1, 5, 2, 6):
                for half in range(2):
                    dma_wqk(f, half, nc.scalar)
            for dt in range(DT):
                dma_wv(dt, nc.gpsimd)
            for f in (3, 7):
                for half in range(2):
                    dma_wqk(f, half, nc.gpsimd)
            for dt in range(DT):
                dma_xt(1, dt, nc.sync)

            # late DMAs issue from inside the period loop (keeps the issuing
            # queues clear for affine_select / exp / y writes early on)
            dma_q = []
            for dt in range(DT):
                dma_q.append((5, lambda dt=dt: dma_xt(2, dt, nc.sync)))
            for hp4 in range(4):
                dma_q.append((6, lambda hp4=hp4: dma_wo(hp4, nc.gpsimd)))
            for dt in range(DT):
                dma_q.append((16, lambda dt=dt: dma_xt(3, dt, nc.sync)))

            # ---- projection group emitters ----
            def v_group(tt):
                def go():
                    ps = ps_mm.tile([128, 512], F32, tag="mm")
                    for dt in range(DT):
                        nc.tensor.matmul(
                            ps,
                            lhsT=xt_all[:, dt, tt * 128:(tt + 1) * 128],
                            rhs=wv_all[:, dt, :],
                            start=(dt == 0),
                            stop=(dt == DT - 1),
                        )
                    nc.vector.tensor_copy(
                        vsb_r[tt // 2][:, tt % 2, :, DH:128],
                        ps.rearrange("p (h c) -> p h c", c=DH),
                    )
                return go

            def qk_group(f, tb):
                def go():
                    ps = ps_mm.tile([128, 512], F32, tag="mm")
                    for dt in range(DT):
                        nc.tensor.matmul(
                            ps,
                            lhsT=wqk_all[:, dt, f * 128:(f + 1) * 128],
                            rhs=xt_all[:, dt, tb * 512:(tb + 1) * 512],
                            start=(dt == 0),
                            stop=(dt == DT - 1),
                        )
                    nc.vector.tensor_copy(qk[f][:, tb * 512:(tb + 1) * 512], ps)
                return go

            def out_group(tt, nb):
                def go():
                    ps = ps_mm.tile([128, 512], F32, tag="mm")
                    for hp4 in range(4):
                        nc.tensor.matmul(
                            ps,
                            lhsT=attn_t[hp4][:, tt * 128:(tt + 1) * 128],
                            rhs=wo_all[:, hp4, nb * 512:(nb + 1) * 512],
                            start=(hp4 == 0),
                            stop=(hp4 == 3),
                        )
                    ysb = stg.tile([128, 512], F32, tag="y", bufs=4,
                                   name=f"ysb{tt}_{nb}")
                    nc.vector.tensor_copy(ysb, ps)
                    nc.sync.dma_start(
                        y_d[tt * 128:(tt + 1) * 128, nb * 512:(nb + 1) * 512],
                        ysb,
                    )
                return go

            # ---- deadline filler queue (projection groups) ----
            deadline_q = []
            for tt in (0, 1):
                deadline_q.append((0, v_group(tt)))
            deadline_q.append((0, qk_group(1, 0)))
            deadline_q.append((0, qk_group(5, 0)))
            for tt in (2, 3):
                deadline_q.append((1, v_group(tt)))
            for j in range(QB):
                for hp in range(4):
                    if j == 0 and hp in (0, 1):
                        continue  # upfront / added above
                    dl = hp_start(j, hp) - 2
                    deadline_q.append((dl, qk_group(hp, j)))
                    deadline_q.append((dl, qk_group(4 + hp, j)))
            for tt in range(4, TT):
                jb = tt // 4
                deadline_q.append((STARTS[jb] + tt // 2 - 1, v_group(tt)))
            deadline_q.sort(key=lambda e: e[0])

            slack_q = []          # (earliest_period, fn, block_j)
            out_ready = [False] * QB
            stages = []           # deferred epilogue stages (None = spacer)

            def period_extras(P):
                while dma_q and dma_q[0][0] <= P:
                    dma_q.pop(0)[1]()
                while deadline_q and deadline_q[0][0] <= P:
                    deadline_q.pop(0)[1]()
                npop = 2 if (len(stages) > 5 or P >= 56) else 1
                for _ in range(npop):
                    if stages:
                        s = stages.pop(0)
                        if s is not None:
                            s()
                if slack_q and slack_q[0][0] <= P and out_ready[slack_q[0][2]]:
                    slack_q.pop(0)[1]()

            def push_epilogue(hp, j, pvA, pvB):
                # pv rows 0:64 = denominator (pre-broadcast by the ones
                # columns), rows 64:128 = o. Copy o out (frees PSUM after
                # the reciprocals, which read the denominator rows straight
                # from PSUM); multiply deferred one period. DVE-only.
                ova = ovp.tile([64, 512], BF16, tag="ov", name=f"ova{hp}_{j}")
                ovb = ovp.tile([64, 512], BF16, tag="ov", name=f"ovb{hp}_{j}")
                recA = stg.tile([64, 512], F32, tag="recA", name=f"recA{hp}_{j}")
                recB = stg.tile([64, 512], F32, tag="recB", name=f"recB{hp}_{j}")
                nc.vector.tensor_copy(ova, pvA[DH:128, :])
                nc.vector.tensor_copy(ovb, pvB[DH:128, :])
                nc.vector.reciprocal_approx_fast(out=recA, in_=pvA[0:DH, :])
                nc.vector.reciprocal_approx_fast(out=recB, in_=pvB[0:DH, :])

                jc = slice(j * 512, (j + 1) * 512)

                def stage2():
                    nc.vector.tensor_mul(attn_t[hp][0:64, jc], ova, recA)
                    nc.vector.tensor_mul(attn_t[hp][64:128, jc], ovb, recB)
                    if hp == 3:
                        out_ready[j] = True

                stages.extend([None, stage2])

            # up-front: only what attention period 0 needs
            qk_group(0, 0)()
            qk_group(4, 0)()

            # ---- attention: q-block j OUTER, head-pair inner, 2 k-tiles
            # per period. The two heads of a pair sit on partitions 0-63 /
            # 64-127 of the same qk tiles, so their K=64 S^T matmuls go to
            # disjoint PE row groups and run concurrently. ----
            P = 0
            for j in range(QB):
                for hp in range(4):
                    qTf = qk[hp]
                    kTf = qk[4 + hp]
                    nkt = 4 * (j + 1)
                    pvA = ps_pv.tile([128, 512], F32, tag="pv")
                    pvB = ps_pv.tile([128, 512], F32, tag="pv")
                    pv_queue = []

                    def pv_mms(kt, pt, q0, pvA=pvA, pvB=pvB, hp=hp, nkt=nkt):
                        def go():
                            nc.tensor.matmul(
                                pvA[:, q0:512],
                                lhsT=vsb_r[kt // 2][:, kt % 2, 2 * hp, :],
                                rhs=pt[:, q0:512],
                                start=(kt == 0), stop=(kt == nkt - 1),
                            )
                            nc.tensor.matmul(
                                pvB[:, q0:512],
                                lhsT=vsb_r[kt // 2][:, kt % 2, 2 * hp + 1, :],
                                rhs=pt[:, 512 + q0:1024],
                                start=(kt == 0), stop=(kt == nkt - 1),
                            )
                        return go

                    def emit_st(kt, j=j, hp=hp, qTf=qTf, kTf=kTf):
                        # diagonal k-tiles: q < 128*(kt-4j) is fully masked --
                        # narrow S^T/exp/mask/PV to the live columns
                        q0 = 128 * (kt - 4 * j) if kt >= 4 * j else 0
                        nq = 512 - q0
                        st = ps_st.tile([128, 1024], F32, tag="st")
                        nc.tensor.matmul(
                            st[:, q0:512],
                            lhsT=kTf[0:64, kt * 128:(kt + 1) * 128],
                            rhs=qTf[0:64, j * 512 + q0:(j + 1) * 512],
                            start=True, stop=True,
                        )
                        nc.tensor.matmul(
                            st[:, 512 + q0:1024],
                            lhsT=kTf[64:128, kt * 128:(kt + 1) * 128],
                            rhs=qTf[64:128, j * 512 + q0:(j + 1) * 512],
                            start=True, stop=True,
                        )
                        pt = ptp.tile([128, 1024], BF16, tag="pt",
                                      name=f"pt{hp}_{j}_{kt}")
                        st_r = st.rearrange("p (h q) -> p h q", h=2)
                        pt_r = pt.rearrange("p (h q) -> p h q", h=2)
                        nc.scalar.activation(
                            pt_r[:, :, q0:512], st_r[:, :, q0:512],
                            mybir.ActivationFunctionType.Exp, scale=0.125
                        )
                        if kt >= 4 * j:  # diagonal k-tile: zero where k > q
                            for half in range(2):
                                nc.gpsimd.affine_select(
                                    out=pt[:, half * 512 + q0:(half + 1) * 512],
                                    in_=pt[:, half * 512 + q0:(half + 1) * 512],
                                    compare_op=mybir.AluOpType.is_ge,
                                    fill=0.0,
                                    base=0,
                                    pattern=[[1, nq]],
                                    channel_multiplier=-1,
                                )
                        return pt, q0

                    for kp in range(nkt // 2):
                        last = kp == nkt // 2 - 1
                        if not last:
                            period_extras(P)
                        for kt in (2 * kp, 2 * kp + 1):
                            pt, q0 = emit_st(kt)
                            pv_queue.append(pv_mms(kt, pt, q0))
                        if last:
                            # flush period: filler between S^T and the full
                            # PV drain gives the last exps time to land
                            period_extras(P)
                            for f_ in pv_queue:
                                f_()
                            pv_queue.clear()
                        else:
                            while len(pv_queue) > 2:
                                pv_queue.pop(0)()
                        P += 1

                    if j == 3 and hp == 3:
                        # drain pending epilogues; start the final pair's
                        # copies+reciprocal immediately; fill its latency
                        # window with held-back out groups (keeps the PE busy
                        # so HAM doesn't throttle); then broadcast+multiply
                        # and the last out-projections
                        while stages:
                            s = stages.pop(0)
                            if s is not None:
                                s()
                        push_epilogue(hp, j, pvA, pvB)
                        stages.pop(0)        # spacer
                        for _, fn, _ in slack_q:
                            fn()
                        slack_q.clear()
                        stages.pop(0)()      # final multiplies
                    else:
                        push_epilogue(hp, j, pvA, pvB)
                    if hp == 3:
                        if j < 3:
                            # out(2)'s last 4 groups are held for the final
                            # chain window (earliest=999 keeps them queued)
                            earliest = {0: 12, 1: 44, 2: 60}[j]
                            for i, (tt, nb) in enumerate(
                                    (tt, nb)
                                    for tt in range(4 * j, 4 * j + 4)
                                    for nb in range(2)):
                                e = earliest + 2 * i
                                if j == 2 and i >= 4:
                                    e = 999
                                slack_q.append((e, out_group(tt, nb), j))
                        else:
                            for tt in range(12, 16):
                                for nb in range(2):
                                    out_group(tt, nb)()

            while stages:
                s = stages.pop(0)
                if s is not None:
                    s()
            while slack_q:
                slack_q.pop(0)[1]()

    nc.compile()
    return nc


def _shard_inputs(x, w_qkv, w_out):
    """Build the 8 per-core input maps (bf16, DMA-block-packed layouts)."""
    bf16 = ml_dtypes.bfloat16
    in_maps = []
    for c in range(8):
        b = c // 2
        hg = c % 2
        q_cols = slice(hg * 512, hg * 512 + 512)
        k_cols = slice(1024 + hg * 512, 1024 + hg * 512 + 512)
        v_cols = slice(2048 + hg * 512, 2048 + hg * 512 + 512)

        xT = np.ascontiguousarray(x[b].T)                    # [1024, 2048]
        # (tb, dt, p, c) blocks
        x_pack = (xT.reshape(8, 128, 4, 512)                 # (dt,p,tb,c)
                  .transpose(2, 0, 1, 3).reshape(4096, 512))

        w_qk = np.concatenate([w_qkv[:, q_cols], w_qkv[:, k_cols]], axis=1)
        # (f, dt, p, c) blocks
        wqk_pack = (w_qk.reshape(8, 128, 8, 128)             # (dt,p,f,c)
                    .transpose(2, 0, 1, 3).reshape(8192, 128))

        in_maps.append({
            "xT": np.ascontiguousarray(x_pack).astype(bf16),
            "w_qk": np.ascontiguousarray(wqk_pack).astype(bf16),
            "w_v": np.ascontiguousarray(w_qkv[:, v_cols]).astype(bf16),
            "w_o": np.ascontiguousarray(
                w_out[hg * 512:hg * 512 + 512, :]).astype(bf16),
        })
    return in_maps


def _run(inputs, trace=False):
    x = np.asarray(inputs["x"], dtype=np.float32)
    w_qkv = np.asarray(inputs["w_qkv"], dtype=np.float32)
    w_out = np.asarray(inputs["w_out"], dtype=np.float32)
    nc = build_kernel()
    in_maps = _shard_inputs(x, w_qkv, w_out)
    res = None
    for attempt in range(3):
        try:
            res = bass_utils.run_bass_kernel_spmd(
                nc, in_maps, core_ids=list(range(8)), trace=trace
            )
            break
        except Exception:
            if attempt == 2:
                raise
    assert res is not None
    out = np.empty((4, T, D), dtype=np.float32)
    for b in range(4):
        out[b] = res.results[2 * b]["y"] + res.results[2 * b + 1]["y"]
    return out, res


def kernel(**inputs):
    out, _ = _run(inputs, trace=False)
    return out
